# revision 27
# baseline (speedup 1.0000x reference)
"""CrossBatchAttention Trainium2 kernel — 8-core tensor-parallel SPMD.

v3: same numerics as v2 (fp8 DoubleRow everywhere), restructured schedule:

  - Merged K/V/Q projection pass: one streaming sweep over the 4 X^T
    quarters computes kt, v and qt together (X loaded once, not 3x).
    The sync DMA queue is free of X traffic during the block phase.
  - Fine-grained startup: quarter-0 X and Wk arrive in 4-k-tile chunks
    consumed by a k-outer loop over 4 live PSUM banks, so the first
    matmul issues as soon as the first 256KB lands.
  - Paired OT AllGathers: heads {0,1} and {2,3} of each batch quarter
    share one AllGather (128KB in / 1MB out) -> 8 collectives instead
    of 16, halving CC-core occupancy.
  - Tight tail: outproj lags its AG by 2 blocks, chunk-3's g1c/RS is
    issued right after the last outproj, and collective triggers are
    ordered so the CC FIFO never parks a ready collective behind an
    unready one.

Quantization (unchanged from v2): X fp8, W* fp8 x64 (/64 on PSUM exit),
qt/kt bf16, P = exp(s/sqrt(d) - 5.0) fp8, ones = 1/8 so rec = 8/den,
otc = O*rec fp8, cacc fp8, g1 partials fp8 x8 through the RS,
sigmoid(logits/64). Host: concat 8 [512,2048] bf16 shards, transpose,
add X -> f32.
"""

import numpy as np
import ml_dtypes

import concourse.bass as bass
import concourse.mybir as mybir
import concourse.tile as tile
from concourse import bacc
from concourse import bass_utils

BF16 = mybir.dt.bfloat16
F32 = mybir.dt.float32
F8 = mybir.dt.float8e4
DR = mybir.MatmulPerfMode.DoubleRow
W_SCALE = 64.0           # all fp8 weights scaled by this on host
O_SCALE = 8.0            # otc = O * 8 (via ones=1/8 in denominator)
G_SCALE = 8.0            # g1 partials carried x8 through the RS
EBIAS = -5.0             # exp(s*SCALE + EBIAS): keeps P in fp8 range

B = 2048
HID = 4096
NH = 32
HD = 128
GH = 1024
NC_ = 8
HPC = NH // NC_          # heads per core = 4
HS = HID // NC_          # hid shard = 512
GS = GH // NC_           # gate-hidden shard = 128
SCALE = 1.0 / float(np.sqrt(HD))

KT_TILES = HID // 128    # 32 k-tiles over the 4096 contraction
KP = KT_TILES // 2       # 16 DoubleRow k-steps
JT = B // 128            # 16 j-tiles over keys
JP = JT // 2             # 8 DoubleRow j-steps
IC = B // 512            # 4 i-chunks of 512 over batch

GELU_FUNC = mybir.ActivationFunctionType.Gelu


def _build_program(allones: bool):
    nc = bacc.Bacc(
        "TRN2",
        target_bir_lowering=False,
        debug=False,
        enable_asserts=False,
        num_devices=NC_,
    )

    # ---- I/O declarations (per-core shapes) ----
    xt_d = nc.dram_tensor("xt", [128, IC, KT_TILES, 512], F8, kind="ExternalInput").ap()
    wq_d = nc.dram_tensor("wq", [128, KT_TILES, HS], F8, kind="ExternalInput").ap()
    wk_d = nc.dram_tensor("wk", [128, KT_TILES, HS], F8, kind="ExternalInput").ap()
    wv_d = nc.dram_tensor("wv", [128, KT_TILES, HS], F8, kind="ExternalInput").ap()
    wo_d = nc.dram_tensor("wo", [128, KT_TILES, HS], F8, kind="ExternalInput").ap()
    # gate W1, X part: full 4096 contraction x this core's 128 gh columns
    gw1x_d = nc.dram_tensor("gw1x", [128, KT_TILES, GS], F8, kind="ExternalInput").ap()
    # fused Wo @ gW1c (attn-output features -> gh), rows in AG perm order
    gwf_d = nc.dram_tensor("gwf", [128, KT_TILES, GS], F8, kind="ExternalInput").ap()
    gw2_d = nc.dram_tensor("gw2", [128, NC_, HS], F8, kind="ExternalInput").ap()
    gb1_d = nc.dram_tensor("gb1", [GS, 1], F32, kind="ExternalInput").ap()
    gb2_d = nc.dram_tensor("gb2", [128, 4], F32, kind="ExternalInput").ap()
    mask01_d = nc.dram_tensor("mask01", [128, JT], BF16, kind="ExternalInput").ap()
    diagm_d = nc.dram_tensor("diagm", [128, 128], F8, kind="ExternalInput").ap()
    out_d = nc.dram_tensor("out", [HS, B], BF16, kind="ExternalOutput").ap()

    groups = [list(range(NC_))]

    with tile.TileContext(nc) as tc:
        with (
            tc.tile_pool(name="persist", bufs=1) as persist,
            tc.tile_pool(name="psum", bufs=1, space="PSUM") as psum,
            tc.tile_pool(name="dram", bufs=1, space="DRAM") as dram,
        ):
            # ---------- persistent SBUF ----------
            qt_sb = persist.tile([128, HPC, B], BF16)     # [d, head, i] 2MB
            kt_sb = persist.tile([128, HPC, B], BF16)     # 2MB
            v_sb = persist.tile([128, JT, HS], F8)        # [j_in, j_tile, hd] 1MB
            mask01_sb = persist.tile([128, JT], BF16)
            diagm_sb = persist.tile([128, 128], F8)
            ones_sb = persist.tile([128, 2, 128], F8)
            ebias_sb = persist.tile([128, 1], F32)
            gb1_sb = persist.tile([GS, 1], F32)
            gb2_sb = persist.tile([128, 4], F32)
            # weights that live through the block phase
            wo_sb = persist.tile([128, KT_TILES, HS], F8)     # 2MB
            gw1x_sb = persist.tile([128, KT_TILES, GS], F8)
            gwf_sb = persist.tile([128, KT_TILES, GS], F8)
            gw2_sb = persist.tile([128, NC_, HS], F8)
            cacc = persist.tile([128, 4, B], F8)
            g1x_sb = persist.tile([128, B], F8)           # gW1x^T X, gh shard

            nc.vector.memset(ones_sb, 1.0 / O_SCALE)
            nc.vector.memset(ebias_sb, EBIAS)

            # ---------- DRAM bounce buffers for collectives ----------
            # paired OT AllGather: rank buffer [2, 128, 512] (heads 2p,2p+1).
            # Chunk 3's pair B is split into two 256-col halves so the tail's
            # last AllGather (and everything behind it) is half-sized.
            ag_in = [[None] * IC for _ in range(2)]
            ag_out = [[None] * IC for _ in range(2)]
            for p in range(2):
                nq = IC if p == 0 else IC - 1
                for q in range(nq):
                    ag_in[p][q] = dram.tile([2, 128, 512], F8,
                                            name=f"ag_in{p}_{q}")
                    ag_out[p][q] = dram.tile(
                        [NC_ * 256, 512], F8, addr_space="Shared",
                        name=f"ag_out{p}_{q}"
                    )
            ag_b3_in, ag_b3_out = [], []
            for hf in range(2):
                ag_b3_in.append(dram.tile([2, 128, 256], F8,
                                          name=f"ag_b3_in{hf}"))
                ag_b3_out.append(dram.tile([NC_ * 256, 256], F8,
                                           addr_space="Shared",
                                           name=f"ag_b3_out{hf}"))
            ag2_in_c, ag2_out_c = [], []
            for icc in range(IC - 1):
                ag2_in_c.append(dram.tile([GS, 512], F8, name=f"ag2_in{icc}"))
                ag2_out_c.append(dram.tile([GH, 512], F8, addr_space="Shared",
                                           name=f"ag2_out{icc}"))
            ag2_3_in, ag2_3_out = [], []
            for hf in range(2):
                ag2_3_in.append(dram.tile([GS, 256], F8, name=f"ag2_3in{hf}"))
                ag2_3_out.append(dram.tile([GH, 256], F8, addr_space="Shared",
                                           name=f"ag2_3out{hf}"))

            # warmups with the same shapes as the real collectives so the
            # first real op doesn't pay the cold-path cost
            warm_ag_i = dram.tile([2, 128, 512], F8)
            warm_ag_o = dram.tile([NC_ * 256, 512], F8, addr_space="Shared")
            warm_ag2_i = dram.tile([GS, 512], F8)
            warm_ag2_o = dram.tile([GH, 512], F8, addr_space="Shared")
            warm_agh_i = dram.tile([2, 128, 256], F8)
            warm_agh_o = dram.tile([NC_ * 256, 256], F8, addr_space="Shared")
            warm_ag2h_i = dram.tile([GS, 256], F8)
            warm_ag2h_o = dram.tile([GH, 256], F8, addr_space="Shared")
            for wi, wo_ in ((warm_ag_i, warm_ag_o), (warm_ag2_i, warm_ag2_o),
                            (warm_agh_i, warm_agh_o),
                            (warm_ag2h_i, warm_ag2h_o)):
                nc.gpsimd.collective_compute(
                    "AllGather", mybir.AluOpType.bypass, replica_groups=groups,
                    ins=[wi.opt()], outs=[wo_.opt()],
                )

            with tc.tile_pool(name="main", bufs=1) as mp:
                # ======== merged K/V/Q projection pass ========
                with tc.tile_pool(name="pkvq", bufs=1) as pkvq:
                    wk_sb = pkvq.tile([128, KT_TILES, HS], F8, tag="wk", bufs=1)
                    wv_sb = pkvq.tile([128, KT_TILES, HS], F8, tag="wv", bufs=1)
                    wq_sb = pkvq.tile([128, KT_TILES, HS], F8, tag="wq", bufs=1)

                    # quarter-0 X and Wk in 4-k-tile chunks (256KB each),
                    # interleaved so the k-outer loop starts ASAP
                    xt_first = pkvq.tile([128, KT_TILES, 512], F8, tag="xt",
                                         bufs=2, name="xt_q")
                    NCH = 8   # chunks of 4 k-tiles
                    for ch in range(NCH):
                        ksl = slice(ch * 4, (ch + 1) * 4)
                        nc.sync.dma_start(out=xt_first[:, ksl, :],
                                          in_=xt_d[:, 0, ksl, :])
                        nc.sync.dma_start(out=wk_sb[:, ksl, :],
                                          in_=wk_d[:, ksl, :])

                    def load_xt(q):
                        xt_q = pkvq.tile([128, KT_TILES, 512], F8, tag="xt",
                                         bufs=2, name="xt_q")
                        nc.sync.dma_start(out=xt_q, in_=xt_d[:, q])
                        return xt_q

                    xt_next = load_xt(1)

                    # remaining weights on the scalar DMA queue so they don't
                    # block the projection-pass X streaming on the sync queue
                    nc.scalar.dma_start(out=wv_sb, in_=wv_d)
                    nc.scalar.dma_start(out=wq_sb, in_=wq_d)
                    nc.scalar.dma_start(out=gw1x_sb, in_=gw1x_d)
                    nc.scalar.dma_start(out=wo_sb, in_=wo_d)
                    nc.scalar.dma_start(out=gwf_sb, in_=gwf_d)
                    nc.scalar.dma_start(out=gw2_sb, in_=gw2_d)
                    if not allones:
                        nc.scalar.dma_start(out=mask01_sb, in_=mask01_d)
                    nc.scalar.dma_start(out=diagm_sb, in_=diagm_d)
                    nc.scalar.dma_start(out=gb1_sb, in_=gb1_d)
                    nc.scalar.dma_start(out=gb2_sb, in_=gb2_d)

                    def proj_dr(wsb, msl, xt_q, ps):
                        for k in range(KP):
                            nc.tensor.matmul(
                                ps,
                                lhsT=wsb[:, 2 * k:2 * k + 2, msl],
                                rhs=xt_q[:, 2 * k:2 * k + 2, :],
                                start=(k == 0),
                                stop=(k == KP - 1),
                                perf_mode=DR,
                            )

                    def v_pass(q, xt_q):
                        for it in range(4):
                            ps = psum.tile([128, 512], F32, tag="mm", bufs=2,
                                           name="ps_v")
                            for k in range(KP):
                                nc.tensor.matmul(
                                    ps,
                                    lhsT=xt_q[:, 2 * k:2 * k + 2,
                                              it * 128:(it + 1) * 128],
                                    rhs=wv_sb[:, 2 * k:2 * k + 2, :],
                                    start=(k == 0),
                                    stop=(k == KP - 1),
                                    perf_mode=DR,
                                )
                            nc.vector.tensor_scalar_mul(
                                v_sb[:, q * 4 + it, :], ps, 1.0 / W_SCALE
                            )

                    def q_pass(q, xt_q):
                        isl = slice(q * 512, (q + 1) * 512)
                        for m in range(4):
                            ps = psum.tile([128, 512], F32, tag="mm", bufs=2,
                                           name="ps_q")
                            proj_dr(wq_sb, slice(m * 128, (m + 1) * 128),
                                    xt_q, ps)
                            nc.vector.tensor_scalar_mul(
                                qt_sb[:, m, isl], ps, 1.0 / W_SCALE
                            )

                    def g1x_pass(q, xt_q):
                        # gW1x^T X for this core's gh shard, full contraction
                        isl = slice(q * 512, (q + 1) * 512)
                        ps = psum.tile([128, 512], F32, tag="mm", bufs=2,
                                       name="ps_g1x")
                        proj_dr(gw1x_sb, slice(0, GS), xt_q, ps)
                        nc.vector.tensor_scalar_mul(
                            g1x_sb[:, isl], ps, 1.0 / W_SCALE
                        )

                    # --- quarter 0: k-outer K pass over 4 live PSUM banks ---
                    # (borrow the scores' "st" tag banks; they're idle here)
                    kpsA = psum.tile([128, 2, 512], F32, tag="st", bufs=2,
                                     name="kpsA")
                    kpsB = psum.tile([128, 2, 512], F32, tag="st", bufs=2,
                                     name="kpsB")
                    kps = [kpsA[:, 0, :], kpsA[:, 1, :],
                           kpsB[:, 0, :], kpsB[:, 1, :]]
                    for ch in range(NCH):
                        for m in range(4):
                            for u in range(2):
                                k = ch * 2 + u
                                nc.tensor.matmul(
                                    kps[m],
                                    lhsT=wk_sb[:, 4 * ch + 2 * u:
                                               4 * ch + 2 * u + 2,
                                               m * 128:(m + 1) * 128],
                                    rhs=xt_first[:, 4 * ch + 2 * u:
                                                 4 * ch + 2 * u + 2, :],
                                    start=(ch == 0 and u == 0),
                                    stop=(ch == NCH - 1 and u == 1),
                                    perf_mode=DR,
                                )
                    for m in range(4):
                        nc.vector.tensor_scalar_mul(
                            kt_sb[:, m, 0:512], kps[m], 1.0 / W_SCALE
                        )
                    v_pass(0, xt_first)
                    q_pass(0, xt_first)
                    g1x_pass(0, xt_first)

                    # --- quarters 1..3: standard m-outer loops ---
                    for q in range(1, IC):
                        xt_q = xt_next
                        if q + 1 < IC:
                            xt_next = load_xt(q + 1)
                        isl = slice(q * 512, (q + 1) * 512)
                        for m in range(4):
                            ps = psum.tile([128, 512], F32, tag="mm", bufs=2,
                                           name="ps_k")
                            proj_dr(wk_sb, slice(m * 128, (m + 1) * 128),
                                    xt_q, ps)
                            nc.vector.tensor_scalar_mul(
                                kt_sb[:, m, isl], ps, 1.0 / W_SCALE
                            )
                        v_pass(q, xt_q)
                        q_pass(q, xt_q)
                        g1x_pass(q, xt_q)

                # ======== interleaved block phase ========
                # block s: (h, q) = (s % 4, s // 4)

                def attention_block(h, q):
                    p = h // 2
                    u = h % 2
                    qsl = slice(q * 512, (q + 1) * 512)
                    pt = mp.tile([128, JT, 512], F8, tag="pt", bufs=2,
                                 name="pt")
                    for jp in range(JP):
                        st = psum.tile([128, 2, 512], F32, tag="st",
                                       bufs=2, name="st")
                        for uu in range(2):
                            j = 2 * jp + uu
                            nc.tensor.matmul(
                                st[:, uu, :],
                                lhsT=kt_sb[:, h, j * 128:(j + 1) * 128],
                                rhs=qt_sb[:, h, qsl],
                                start=True,
                                stop=True,
                            )
                        nc.scalar.activation(
                            pt[:, 2 * jp:2 * jp + 2, :],
                            st,
                            mybir.ActivationFunctionType.Exp,
                            bias=ebias_sb,
                            scale=SCALE,
                        )
                        for uu in range(2):
                            j = 2 * jp + uu
                            if not allones:
                                nc.vector.tensor_scalar_mul(
                                    pt[:, j, :], pt[:, j, :],
                                    mask01_sb[:, j:j + 1],
                                )
                            if j // 4 == q:
                                c0 = (j % 4) * 128
                                nc.vector.tensor_mul(
                                    pt[:, j, c0:c0 + 128],
                                    pt[:, j, c0:c0 + 128],
                                    diagm_sb,
                                )
                    den_ps = psum.tile([128, 512], F32, tag="acc", bufs=2,
                                       name="den_ps")
                    for jp in range(JP):
                        nc.tensor.matmul(
                            den_ps,
                            lhsT=ones_sb,
                            rhs=pt[:, 2 * jp:2 * jp + 2, :],
                            start=(jp == 0),
                            stop=(jp == JP - 1),
                            perf_mode=DR,
                        )
                    ot_ps = psum.tile([128, 512], F32, tag="acc", bufs=2,
                                      name="ot_ps")
                    for jp in range(JP):
                        nc.tensor.matmul(
                            ot_ps,
                            lhsT=v_sb[:, 2 * jp:2 * jp + 2,
                                      h * 128:(h + 1) * 128],
                            rhs=pt[:, 2 * jp:2 * jp + 2, :],
                            start=(jp == 0),
                            stop=(jp == JP - 1),
                            perf_mode=DR,
                        )
                    rec = mp.tile([128, 512], F32, tag="rec", bufs=1)
                    nc.vector.reciprocal_approx_fast(out=rec, in_=den_ps)
                    otc = mp.tile([128, 512], F8, tag="otc", bufs=1)
                    nc.vector.tensor_mul(otc, ot_ps, rec)
                    if p == 1 and q == IC - 1:
                        # chunk 3 pair B: split into two half-width AGs
                        for hf in range(2):
                            nc.sync.dma_start(
                                out=ag_b3_in[hf][u],
                                in_=otc[:, hf * 256:(hf + 1) * 256],
                            )
                        if u == 1:
                            for hf in range(2):
                                nc.gpsimd.collective_compute(
                                    "AllGather",
                                    mybir.AluOpType.bypass,
                                    replica_groups=groups,
                                    ins=[ag_b3_in[hf].opt()],
                                    outs=[ag_b3_out[hf].opt()],
                                )
                    else:
                        nc.sync.dma_start(out=ag_in[p][q][u], in_=otc)
                        if u == 1:
                            nc.gpsimd.collective_compute(
                                "AllGather",
                                mybir.AluOpType.bypass,
                                replica_groups=groups,
                                ins=[ag_in[p][q].opt()],
                                outs=[ag_out[p][q].opt()],
                            )

                def otg_load(p, ic):
                    otg = mp.tile([128, 2 * NC_, 512], F8, tag="otg", bufs=2,
                                  name="otg")
                    nc.sync.dma_start(
                        out=otg,
                        in_=ag_out[p][ic].rearrange("(g j) i -> j g i", j=128),
                    )
                    return otg

                def otg_load_b3(hf):
                    otg = mp.tile([128, 2 * NC_, 256], F8, tag="otg", bufs=2,
                                  name="otgh")
                    nc.sync.dma_start(
                        out=otg,
                        in_=ag_b3_out[hf].rearrange("(g j) i -> j g i", j=128),
                    )
                    return otg

                def outproj_pair(p, c0, w, otg):
                    # contraction over the 16 gathered head-tiles of pair p,
                    # for output batch columns [c0, c0+w)
                    csl = slice(c0, c0 + w)
                    for m in range(4):
                        ps = psum.tile([128, w], F32, tag="mm", bufs=2,
                                       name="ps_wo")
                        for r in range(NC_):
                            nc.tensor.matmul(
                                ps,
                                lhsT=wo_sb[:, p * 16 + 2 * r:
                                           p * 16 + 2 * r + 2,
                                           m * 128:(m + 1) * 128],
                                rhs=otg[:, 2 * r:2 * r + 2, :],
                                start=(r == 0),
                                stop=(r == NC_ - 1),
                                perf_mode=DR,
                            )
                        if p == 0:
                            nc.vector.tensor_scalar_mul(
                                cacc[:, m, csl], ps, 1.0 / (W_SCALE * O_SCALE)
                            )
                        else:
                            nc.vector.scalar_tensor_tensor(
                                cacc[:, m, csl], ps, 1.0 / (W_SCALE * O_SCALE),
                                cacc[:, m, csl],
                                op0=mybir.AluOpType.mult,
                                op1=mybir.AluOpType.add,
                            )

                def g1_part(half, otg, w, start):
                    # one 2048-row half of the gWf^T @ otg contraction
                    ps = psum.tile([128, w], F32, tag="mm", bufs=2,
                                   name="ps_g1")
                    for r in range(NC_):
                        nc.tensor.matmul(
                            ps,
                            lhsT=gwf_sb[:, half * 16 + 2 * r:
                                        half * 16 + 2 * r + 2, :],
                            rhs=otg[:, 2 * r:2 * r + 2, :],
                            start=(r == 0),
                            stop=(r == NC_ - 1),
                            perf_mode=DR,
                        )
                    return ps

                def g1_finish(ps_or_pre, c0, w, ag2i, ag2o):
                    gt_ch = mp.tile([128, w], F8, tag="gt", bufs=1)
                    nc.scalar.activation(gt_ch, ps_or_pre, GELU_FUNC,
                                         bias=gb1_sb, scale=1.0)
                    nc.sync.dma_start(out=ag2i, in_=gt_ch)
                    nc.gpsimd.collective_compute(
                        "AllGather",
                        mybir.AluOpType.bypass,
                        replica_groups=groups,
                        ins=[ag2i.opt()],
                        outs=[ag2o.opt()],
                    )

                def g1_chunk(ic, otg_a, otg_b):
                    # this core's gh-shard of g1 for the chunk, full local
                    # contraction: gWf^T @ otg (= gW1c^T cross) + g1X;
                    # then gelu and AllGather of the activated shard.
                    csl = slice(ic * 512, (ic + 1) * 512)
                    ps = psum.tile([128, 512], F32, tag="mm", bufs=2,
                                   name="ps_g1")
                    for r in range(NC_):
                        nc.tensor.matmul(
                            ps,
                            lhsT=gwf_sb[:, 2 * r:2 * r + 2, :],
                            rhs=otg_a[:, 2 * r:2 * r + 2, :],
                            start=(r == 0),
                            stop=False,
                            perf_mode=DR,
                        )
                    for r in range(NC_):
                        nc.tensor.matmul(
                            ps,
                            lhsT=gwf_sb[:, 16 + 2 * r:16 + 2 * r + 2, :],
                            rhs=otg_b[:, 2 * r:2 * r + 2, :],
                            start=False,
                            stop=(r == NC_ - 1),
                            perf_mode=DR,
                        )
                    g1pre = mp.tile([128, 512], BF16, tag="g1pre", bufs=1)
                    nc.vector.scalar_tensor_tensor(
                        g1pre, ps, 1.0 / (W_SCALE * O_SCALE),
                        g1x_sb[:, csl],
                        op0=mybir.AluOpType.mult,
                        op1=mybir.AluOpType.add,
                    )
                    g1_finish(g1pre, ic * 512, 512, ag2_in_c[ic],
                              ag2_out_c[ic])

                def gtf_load(ic):
                    gtf = mp.tile([128, NC_, 512], F8, tag="gtf",
                                  bufs=1, name="gtf")
                    nc.sync.dma_start(
                        out=gtf,
                        in_=ag2_out_c[ic].rearrange("(r p) i -> p r i", p=128),
                    )
                    return gtf

                def gtf_load_3(hf):
                    gtf = mp.tile([128, NC_, 256], F8, tag="gtf",
                                  bufs=1, name="gtfh")
                    nc.sync.dma_start(
                        out=gtf,
                        in_=ag2_3_out[hf].rearrange("(r p) i -> p r i", p=128),
                    )
                    return gtf

                def gate_chain(c0, w, gtf):
                    csl = slice(c0, c0 + w)
                    for m in range(4):
                        ps = psum.tile([128, w], F32, tag="mm", bufs=2,
                                       name="ps_gw2")
                        for r in range(NC_ // 2):
                            nc.tensor.matmul(
                                ps,
                                lhsT=gw2_sb[:, 2 * r:2 * r + 2,
                                            m * 128:(m + 1) * 128],
                                rhs=gtf[:, 2 * r:2 * r + 2, :],
                                start=(r == 0),
                                stop=(r == NC_ // 2 - 1),
                                perf_mode=DR,
                            )
                        gate_ch = mp.tile([128, w], BF16, tag="gate",
                                          bufs=2)
                        nc.scalar.activation(
                            gate_ch, ps,
                            mybir.ActivationFunctionType.Sigmoid,
                            bias=gb2_sb[:, m:m + 1], scale=1.0 / W_SCALE,
                        )
                        outt = mp.tile([128, w], BF16, tag="outt", bufs=1)
                        nc.vector.tensor_mul(outt, gate_ch, cacc[:, m, csl])
                        nc.sync.dma_start(
                            out=out_d[m * 128:(m + 1) * 128, csl], in_=outt
                        )

                # schedule (AG wall ~18us > 1 block, so lag loads by 2):
                #   otg_A(ic) @ 4ic+3   outproj_A(ic) @ 4ic+4
                #   otg_B(ic) @ 4ic+5   outproj_B + g1_chunk(ic) @ 4ic+6
                #   gtf(ic) @ 4ic+8     gate(ic) @ 4ic+9
                otg_pend = {}
                gtf_pend = {}
                for s in range(16):
                    h, q = s % 4, s // 4
                    attention_block(h, q)
                    r4 = s % 4
                    if r4 == 3:
                        otg_pend[(0, q)] = otg_load(0, q)
                    if r4 == 0 and s >= 4:
                        outproj_pair(0, (q - 1) * 512, 512,
                                     otg_pend[(0, q - 1)])
                    if r4 == 1 and s >= 5:
                        otg_pend[(1, q - 1)] = otg_load(1, q - 1)
                    if r4 == 2 and s >= 6:
                        ic = q - 1
                        otg_a = otg_pend.pop((0, ic))
                        otg_b = otg_pend.pop((1, ic))
                        outproj_pair(1, ic * 512, 512, otg_b)
                        g1_chunk(ic, otg_a, otg_b)
                    if r4 == 0 and s >= 8:
                        ic = (s - 8) // 4
                        gtf_pend[ic] = gtf_load(ic)
                    if r4 == 1 and s >= 9:
                        ic = (s - 9) // 4
                        gate_chain(ic * 512, 512, gtf_pend.pop(ic))

                # ---- tail: chunk 3 (pair A full width, pair B split in two
                # half-width chains) + chunk 2/3 gate chains ----
                otg_a3 = otg_pend.pop((0, 3))
                outproj_pair(0, 1536, 512, otg_a3)
                gtf_pend[2] = gtf_load(2)
                # A-part of g1 for both halves (frees otg_a3 before the
                # half otg loads rotate its buffer slot)
                tmp = []
                for hf in range(2):
                    psA = g1_part(0, otg_a3[:, :, hf * 256:(hf + 1) * 256],
                                  256, True)
                    t = mp.tile([128, 256], BF16, tag="g1tmp", bufs=2)
                    nc.vector.scalar_tensor_tensor(
                        t, psA, 1.0 / (W_SCALE * O_SCALE),
                        g1x_sb[:, 1536 + hf * 256:1536 + (hf + 1) * 256],
                        op0=mybir.AluOpType.mult,
                        op1=mybir.AluOpType.add,
                    )
                    tmp.append(t)
                gate_chain(1024, 512, gtf_pend.pop(2))
                for hf in range(2):
                    otg_b = otg_load_b3(hf)
                    outproj_pair(1, 1536 + hf * 256, 256, otg_b)
                    psB = g1_part(1, otg_b, 256, True)
                    g1pre = mp.tile([128, 256], BF16, tag="g1pre", bufs=1)
                    nc.vector.scalar_tensor_tensor(
                        g1pre, psB, 1.0 / (W_SCALE * O_SCALE), tmp[hf],
                        op0=mybir.AluOpType.mult,
                        op1=mybir.AluOpType.add,
                    )
                    g1_finish(g1pre, 1536 + hf * 256, 256,
                              ag2_3_in[hf], ag2_3_out[hf])
                for hf in range(2):
                    gtf3 = gtf_load_3(hf)
                    gate_chain(1536 + hf * 256, 256, gtf3)

    nc.compile()
    return nc


def _q8(x, scale=1.0):
    f8 = ml_dtypes.float8_e4m3
    return np.ascontiguousarray(
        np.clip(np.asarray(x, dtype=np.float32) * scale, -240.0, 240.0)
    ).astype(f8)


def _make_in_maps(inputs):
    f32 = np.float32
    X = np.asarray(inputs["hidden_states"], dtype=f32)
    mask = np.asarray(inputs["attention_mask"])
    Wq = np.asarray(inputs["Wq"], dtype=f32)
    Wk = np.asarray(inputs["Wk"], dtype=f32)
    Wv = np.asarray(inputs["Wv"], dtype=f32)
    Wo = np.asarray(inputs["Wo"], dtype=f32)
    gW1 = np.asarray(inputs["gW1"], dtype=f32)
    gb1 = np.asarray(inputs["gb1"], dtype=f32)
    gW2 = np.asarray(inputs["gW2"], dtype=f32)
    gb2 = np.asarray(inputs["gb2"], dtype=f32)

    XT8 = _q8(X.T)                                       # [4096, 2048]
    # pre-tile to [partition, quarter, k-tile, 512] so every DMA moves
    # large contiguous per-partition segments
    XTT = np.ascontiguousarray(
        XT8.reshape(KT_TILES, 128, IC, 512).transpose(1, 2, 0, 3))

    def _tile_w(w8):  # [K, M] -> [128, K/128, M]
        kt = w8.shape[0] // 128
        return np.ascontiguousarray(
            w8.reshape(kt, 128, w8.shape[1]).transpose(1, 0, 2))
    # Wo row permutation to match the paired per-head AllGather chunk
    # assembly: OT_full row (p*2048 + (2r+u)*128 + d) holds global head
    # (4r + 2p + u), dim d.
    perm = np.empty(HID, dtype=np.int64)
    for p in range(2):
        for r in range(NC_):
            for u in range(2):
                g = 4 * r + 2 * p + u
                dst = p * 2048 + (2 * r + u) * 128
                perm[dst:dst + 128] = np.arange(g * 128, (g + 1) * 128)
    Wo_p = Wo[perm]
    mask01_t = np.ascontiguousarray(
        mask.astype(f32).reshape(JT, 128).T).astype(ml_dtypes.bfloat16)
    diagm = _q8(1.0 - np.eye(128, dtype=f32))

    # fused Wo @ gW1c: attention-output features (AG perm order) -> gh
    Wf_p = Wo_p @ gW1[HID:]                              # [4096, 1024]
    gW1x = gW1[:HID]                                     # [4096, 1024]

    in_maps = []
    for c in range(NC_):
        hsl = slice(c * HS, (c + 1) * HS)
        gsl = slice(c * GS, (c + 1) * GS)
        in_maps.append({
            "xt": XTT,
            "wq": _tile_w(_q8(Wq[:, hsl], W_SCALE)),
            "wk": _tile_w(_q8(Wk[:, hsl], W_SCALE)),
            "wv": _tile_w(_q8(Wv[:, hsl], W_SCALE)),
            "wo": _tile_w(_q8(Wo_p[:, hsl], W_SCALE)),
            "gw1x": _tile_w(_q8(gW1x[:, gsl], W_SCALE)),
            "gwf": _tile_w(_q8(Wf_p[:, gsl], W_SCALE)),
            "gw2": _tile_w(_q8(gW2[:, hsl], W_SCALE)),
            "gb1": np.ascontiguousarray(gb1[gsl].reshape(GS, 1)),
            "gb2": np.ascontiguousarray(gb2[hsl].reshape(4, 128).T),
            "mask01": mask01_t,
            "diagm": diagm,
        })
    return in_maps


_NC_CACHE = {}


def _run(inputs, trace=False):
    allones = bool(np.asarray(inputs["attention_mask"]).all())
    nc = _NC_CACHE.get(allones)
    if nc is None:
        nc = _build_program(allones)
        _NC_CACHE[allones] = nc
    in_maps = _make_in_maps(inputs)
    res = bass_utils.run_bass_kernel_spmd(
        nc, in_maps, core_ids=list(range(NC_)), trace=trace
    )
    shards = [np.asarray(res.results[c]["out"], dtype=np.float32)
              for c in range(NC_)]
    gated = np.concatenate(shards, axis=0).T  # gate * cross, [2048, 4096]
    out = np.asarray(inputs["hidden_states"], dtype=np.float32) + gated
    return np.ascontiguousarray(out), res


def kernel(**inputs) -> np.ndarray:
    out, _ = _run(inputs, trace=False)
    return out
